# revision 29
# baseline (speedup 1.0000x reference)
"""Trainium2 Bass kernel for a dense multi-head attention layer.

Problem: B=4, S=2048, D=1024, H=16, DH=64 attention (QKV projections +
softmax(QK^T/sqrt(DH))V), fp32 reference, attention_mask all-ones, zero
biases.

Sharding (8 NeuronCores): core c handles batch b=c//2 and head-half
hh=c%2 (8 of 16 heads).  Per-core work is perfectly balanced with no
collectives: each core projects its 8 heads' Q/K/V over the full
sequence of its batch and runs attention for those heads.

Per-core device algorithm (all matmuls bf16 in / fp32 PSUM accumulate):
  - Q^T, K^T computed in [outcol, token] layout (lhsT = W, rhs = X^T).
  - V computed in [token, outcol] layout (lhsT = X^T tile, rhs = W),
    stored per (t_tile, head) with a constant ones column appended.
  - scores^T[t, f] per head via lhsT=K^T tile (contraction dh=64),
    heads sequential (measured: tile_position row-packing concurrency
    was a net loss on HW).
  - exp via ScalarE activation (scale=1/8 fused) straight out of PSUM,
    3 score tiles (both heads mixed, subtile-linear order) per
    activation to amortize the ~0.5us per-instruction ScalarE overhead
    -> bf16 expT in SBUF.  No max-subtraction (scores are O(1);
    softmax is shift-invariant).
  - PV: ctx^T[dh,f] = sum_t V[t,dh]*expT[t,f] with lhsT=[V|1] (M=65);
    row 64 accumulates the softmax denominator for free.
  - normalize: reciprocal of the denominator row, DRAM-bounce broadcast
    across 64 partitions, DVE multiply; DMA out as ctx^T [512, 2048].
Host reassembles: out[b, :, hh*512:(hh+1)*512] = core_out.T
"""

import numpy as np
import ml_dtypes

B, S, D = 4, 2048, 1024
H, DH = 16, 64
NCORES = 8
HL = 8            # local heads per core
OC = HL * DH      # 512 local output columns
P = 128
NDC = D // P      # 8 contraction chunks for projections
FB = 512          # f-block (query) width
NFB = S // FB     # 4
NTT = S // P      # 16 key tiles
SCALE = 1.0 / np.sqrt(DH)

_CACHE = {}


def _build_nc(repeat=None, variant=None):
    import contextlib
    import concourse.bass as bass
    import concourse.tile as tile
    from concourse import bacc, mybir
    from concourse.bass import ts, ds

    bf16 = mybir.dt.bfloat16
    f32 = mybir.dt.float32
    Exp = mybir.ActivationFunctionType.Exp

    nc = bacc.Bacc("TRN2", target_bir_lowering=False, debug=False)

    xfT_d = nc.dram_tensor("xfT", [D, S], bf16, kind="ExternalInput")
    xtT_d = nc.dram_tensor("xtT", [D, S], bf16, kind="ExternalInput")
    wq_d = nc.dram_tensor("wq", [D, OC], bf16, kind="ExternalInput")
    wk_d = nc.dram_tensor("wk", [D, OC], bf16, kind="ExternalInput")
    wv_d = nc.dram_tensor("wv", [D, OC], bf16, kind="ExternalInput")
    out_d = nc.dram_tensor("out", [S, OC], f32, kind="ExternalOutput")

    # groups of t_tiles per ScalarE activation, within each 8-tile half
    HGROUPS = [(0, 2), (2, 2), (4, 2), (6, 2)]

    with tile.TileContext(nc) as tc:
        with (
            tc.tile_pool(name="persist", bufs=1) as pp,
            tc.tile_pool(name="proj_in", bufs=1) as pin,
            tc.tile_pool(name="expt", bufs=1 if variant in ("pvonly", "pvnn") else 3) as ep,
            tc.tile_pool(name="small", bufs=2) as sp,
            tc.tile_pool(name="ps_sc2", bufs=2, space="PSUM") as ps_sc2,
            tc.tile_pool(name="ps_pv", bufs=2, space="PSUM") as ps_pv,
            tc.tile_pool(name="ps_b1", bufs=2, space="PSUM") as ps_b1,
        ):
            qT = pp.tile([P, 4, S], bf16, tag="qT")
            kT = pp.tile([P, 4, S], bf16, tag="kT")
            v = pp.tile([P, NTT, HL, DH + 1], bf16, tag="v")
            nc.vector.memset(v[:, :, :, DH], 1.0)
            ones64 = pp.tile([1, DH], f32, tag="ones64")
            nc.vector.memset(ones64[:], 1.0)
            eC = None
            if variant in ("pvonly", "pvnn"):
                eC = pp.tile([P, 16, FB], bf16, tag="eC")
                nc.vector.memset(eC[:], 0.001)

            xfT = pin.tile([P, NDC, S], bf16, tag="xfT")
            xtT = pin.tile([P, NDC, S], bf16, tag="xtT")
            wq = pin.tile([P, NDC, OC], bf16, tag="wq")
            wk = pin.tile([P, NDC, OC], bf16, tag="wk")
            wv = pin.tile([P, NDC, OC], bf16, tag="wv")
            def dma_w(sb_t, dr):
                nc.sync.dma_start(
                    out=sb_t[:],
                    in_=dr.ap().rearrange("(c p) n -> p c n", p=P),
                )

            def dma_x(sb_t, dr, tch):
                nc.sync.dma_start(
                    out=sb_t[:, :, ts(tch, FB)],
                    in_=dr.ap()[:, ts(tch, FB)].rearrange(
                        "(c p) n -> p c n", p=P),
                )

            # input DMAs chunked and ordered by first use so the prefix
            # projection chains start ~6us in instead of waiting ~31us for
            # the whole 11MB load.
            dma_w(wk, wk_d)
            dma_x(xtT, xtT_d, 0)
            dma_x(xtT, xtT_d, 1)
            dma_w(wq, wq_d)
            dma_x(xfT, xfT_d, 0)
            dma_w(wv, wv_d)
            dma_x(xtT, xtT_d, 2)
            dma_x(xtT, xtT_d, 3)
            dma_x(xfT, xfT_d, 1)
            dma_x(xfT, xfT_d, 2)
            dma_x(xfT, xfT_d, 3)

            def proj_chain(w_sb, x_sb, dst, ot, tch):
                psq = ps_b1.tile([P, FB], f32, tag="b1")
                for dc in range(NDC):
                    nc.tensor.matmul(
                        psq[:],
                        w_sb[:, dc, ts(ot, P)],
                        x_sb[:, dc, ts(tch, FB)],
                        start=(dc == 0),
                        stop=(dc == NDC - 1),
                    )
                nc.vector.tensor_copy(dst[:, ot, ts(tch, FB)], psq[:])

            def Kc(ot, tch):
                return lambda: proj_chain(wk, xtT, kT, ot, tch)

            def Qc(ot, tch):
                return lambda: proj_chain(wq, xfT, qT, ot, tch)

            def proj_v_one(tt):
                psv = ps_b1.tile([P, FB], f32, tag="b1")
                for dc in range(NDC):
                    nc.tensor.matmul(
                        psv[:],
                        xtT[:, dc, ts(tt, P)],
                        wv[:, dc, :],
                        start=(dc == 0),
                        stop=(dc == NDC - 1),
                    )
                nc.vector.tensor_copy(
                    v[:, tt, :, 0:DH],
                    psv[:].rearrange("p (h d) -> p h d", h=HL),
                )

            def Vc(tt):
                return lambda: proj_v_one(tt)

            def scores_half(j, fb, half, e):
                """scores^T + exp for t_tiles [8*half, 8*half+8), both heads
                of pair j, emitted strictly head-alternating [A0 B0 A1 B1
                ...] so consecutive MMs occupy different PE row groups
                (64x128 row tiling -> 2 concurrent MMs) and different PSUM
                banks; one ScalarE activation per 3-subtile slot."""
                if variant in ("pvonly", "pvnn"):
                    return
                order = []
                for q in range(4):
                    order += [(0, 2 * q), (0, 2 * q + 1),
                              (1, 2 * q), (1, 2 * q + 1)]
                bounds = [(2 * g, 2, ps_sc2) for g in range(8)]
                for gi, (start_s, glen, pool) in enumerate(bounds):
                    sc = pool.tile([P, glen, FB], f32, tag="sc")
                    for t in range(glen):
                        s = start_s + t
                        if variant == "quad":
                            hh_, i = order[s]
                        else:
                            hh_, i = s % 2, s // 2
                        tt = half * 8 + i
                        base = 0 if variant == "packoff" else hh_ * 64
                        if variant != "noscores":
                            nc.tensor.matmul(
                                sc[:, t, :],
                                kT[base:base + 64, j, ts(tt, P)],
                                qT[base:base + 64, j, ts(fb, FB)],
                                start=True, stop=True,
                                tile_position=(base, 0),
                            )
                    if variant == "half_act" and gi % 2:
                        continue
                    if variant in ("noact", "noscores"):
                        continue
                    nc.scalar.activation(
                        e[:, start_s:start_s + glen, :], sc[:, 0:glen, :],
                        Exp, scale=float(SCALE),
                    )

            def pv_full(cpsA, cpsB, j, e0, e1):
                """PV in ctx[f, dh] layout: lhsT = expT tile (128 cols ->
                FWL), rhs = [V|1] (streams 65 cols); psum col 64 of each
                f-tile block accumulates the softmax denominator, which
                lands per-PARTITION (f), so normalize is a native
                tensor_scalar.  The 4 f-tile chains share a PSUM bank, and
                start=True clears has_written BANK-wide, so each chain's 16
                MMs must run contiguously; A/B heads alternate banks."""
                if variant in ("nopv", "noact", "noscores"):
                    return
                if variant in ("pvonly", "pvnn"):
                    e0 = e1 = eC
                for ft in range(4):
                    for tt in range(NTT):
                        e = e0 if tt < 8 else e1
                        i = tt % 8
                        for hh_, cps in ((0, cpsA), (1, cpsB)):
                            nc.tensor.matmul(
                                cps[:, ft, :],
                                e[:, 2 * i + hh_, ts(ft, P)],
                                v[:, tt, 2 * j + hh_, :],
                                start=(tt == 0),
                                stop=(tt == NTT - 1),
                            )

            def norm_flip(cps, hl, fb):
                """Normalize ctx[f, dh] by the per-partition denominator
                (psum col 64 of each f-tile block) and DMA out in the
                natural [f, head*64+dh] layout."""
                if variant in ("nopv", "noact", "noscores", "pvnn"):
                    return
                recip = sp.tile([P, 4], f32, tag="recip")
                nc.vector.reciprocal(recip[:], cps[:, :, DH])
                outst = sp.tile([P, 4, DH], f32, tag="outst")
                for ft in range(4):
                    nc.vector.tensor_scalar_mul(
                        outst[:, ft, :], cps[:, ft, 0:DH],
                        recip[:, ft:ft + 1])
                nc.sync.dma_start(
                    out=out_d.ap()[ts(fb, FB), ds(hl * DH, DH)].rearrange(
                        "(ft p) d -> p ft d", p=P),
                    in_=outst[:],
                )

            rep_ctx = (
                tc.For_i(0, repeat, 1) if repeat else contextlib.nullcontext()
            )

            def finish_round(prev):
                """PV + normalize of the previous round -- emitted after
                the NEXT round's half-0 scores so ScalarE always has fresh
                score groups while TensorE runs the PV block (round-
                granularity software pipeline)."""
                e0, e1, j, fb = prev
                cpsA = ps_pv.tile([P, 4, DH + 1], f32, tag="pv")
                cpsB = ps_pv.tile([P, 4, DH + 1], f32, tag="pv")
                pv_full(cpsA, cpsB, j, e0, e1)
                norm_flip(cpsA, 2 * j, fb)
                norm_flip(cpsB, 2 * j + 1, fb)

            def attn_round(j, fb, prev, f0=(), f1=(), pre_scored=None):
                """One (pair, f-block) round, software-pipelined: emits
                half-0 scores, the PREVIOUS round's second-half PV+norm,
                half-0 PV, half-1 scores; returns the pending second half.
                f0 fillers run on TensorE while ScalarE drains the half-0
                exps; fend fillers at round end.  Fillers must not be placed
                between the cpsA alloc and the last norm (ps_b1 ring)."""
                if prev is not None and variant == "pvfirst":
                    finish_round(prev)
                if pre_scored is None:
                    e0 = ep.tile([P, 16, FB], bf16, tag="e")
                    scores_half(j, fb, 0, e0)
                else:
                    e0 = pre_scored
                if prev is not None and variant != "pvfirst":
                    finish_round(prev)
                for f in f0:
                    f()
                e1 = ep.tile([P, 16, FB], bf16, tag="e")
                scores_half(j, fb, 1, e1)
                for f in f1:
                    f()
                return (e0, e1, j, fb)

            # emission order = program order for Tile's dependency tracking,
            # and also the scheduling priority.  Get ScalarE started as early
            # as possible (scores of pair 0 need only qT/kT tile 0), then
            # spread the projection chains across the rounds so TensorE has
            # filler work exactly where it would otherwise stall on the
            # ScalarE exp drain (sc-pool buffers).
            with rep_ctx:
                # minimal prefix for scores(0,0,half0): kT tile0 t-chunks
                # 0-1 and qT tile0 chunk 0
                proj_chain(wk, xtT, kT, 0, 0)
                proj_chain(wk, xtT, kT, 0, 1)
                proj_chain(wq, xfT, qT, 0, 0)
                e00 = ep.tile([P, 16, FB], bf16, tag="e")
                scores_half(0, 0, 0, e00)
                proj_chain(wk, xtT, kT, 0, 2)
                proj_chain(wk, xtT, kT, 0, 3)
                for tt in range(4):
                    proj_v_one(tt)
                fills = {(0, 0): [Qc(0, 1)] + [Vc(t) for t in range(4, 9)],
                         (3, 3): []}
                f1s = {(0, 0): [Vc(t) for t in range(9, 16)]}
                for j in range(4):
                    if j < 3:
                        fills[(j, 1)] = [Kc(j + 1, 0), Qc(j, 2)]
                        fills[(j, 2)] = [Kc(j + 1, 1), Qc(j, 3)]
                        fills[(j, 3)] = [Qc(j + 1, 0), Kc(j + 1, 2)]
                        fills[(j + 1, 0)] = [Kc(j + 1, 3), Qc(j + 1, 1)]
                    else:
                        fills.setdefault((j, 1), [Qc(j, 2)])
                        fills.setdefault((j, 2), [Qc(j, 3)])
                prev = None
                for j in range(4):
                    for fb in range(NFB):
                        prev = attn_round(
                            j, fb, prev, f0=fills[(j, fb)],
                            f1=f1s.get((j, fb), ()),
                            pre_scored=e00 if (j, fb) == (0, 0) else None,
                        )
                finish_round(prev)
    nc.compile()
    return nc


def _get_nc():
    if "nc" not in _CACHE:
        _CACHE["nc"] = _build_nc()
    return _CACHE["nc"]


def _numpy_reference(x_from, x_to, attention_mask, wq, bq, wk, bk, wv, bv):
    """General fallback (used only if mask/biases are not the expected
    all-ones / zeros of this problem instance)."""
    b, fs, _ = x_from.shape
    ts_ = x_to.shape[1]
    q = (x_from @ wq + bq).reshape(b, fs, H, DH).transpose(0, 2, 1, 3)
    k = (x_to @ wk + bk).reshape(b, ts_, H, DH).transpose(0, 2, 1, 3)
    v = (x_to @ wv + bv).reshape(b, ts_, H, DH).transpose(0, 2, 1, 3)
    scores = np.einsum("bhfd,bhtd->bhft", q, k) * (1.0 / np.sqrt(DH))
    adder = (1.0 - attention_mask[:, None, :, :].astype(np.float32)) * -10000.0
    scores = scores + adder
    scores -= scores.max(axis=-1, keepdims=True)
    e = np.exp(scores)
    probs = e / e.sum(axis=-1, keepdims=True)
    ctx = np.einsum("bhft,bhtd->bhfd", probs, v)
    return ctx.transpose(0, 2, 1, 3).reshape(b, fs, H * DH).astype(np.float32)


def _make_in_maps(x_from, x_to, wq, wk, wv):
    bf = ml_dtypes.bfloat16
    xfT = [np.ascontiguousarray(x_from[b].T).astype(bf) for b in range(B)]
    xtT = [np.ascontiguousarray(x_to[b].T).astype(bf) for b in range(B)]
    wq_h = [np.ascontiguousarray(wq[:, hh * OC:(hh + 1) * OC]).astype(bf)
            for hh in range(2)]
    wk_h = [np.ascontiguousarray(wk[:, hh * OC:(hh + 1) * OC]).astype(bf)
            for hh in range(2)]
    wv_h = [np.ascontiguousarray(wv[:, hh * OC:(hh + 1) * OC]).astype(bf)
            for hh in range(2)]
    in_maps = []
    for c in range(NCORES):
        b, hh = c // 2, c % 2
        in_maps.append({
            "xfT": xfT[b], "xtT": xtT[b],
            "wq": wq_h[hh], "wk": wk_h[hh], "wv": wv_h[hh],
        })
    return in_maps


def _assemble(results):
    out = np.empty((B, S, H * DH), np.float32)
    for c in range(NCORES):
        b, hh = c // 2, c % 2
        out[b, :, hh * OC:(hh + 1) * OC] = results[c]["out"]
    return out


def _run(inputs, **spmd_kwargs):
    x_from = np.asarray(inputs["x_from"], dtype=np.float32)
    x_to = np.asarray(inputs["x_to"], dtype=np.float32)
    mask = np.asarray(inputs["attention_mask"])
    wq = np.asarray(inputs["wq"], dtype=np.float32)
    wk = np.asarray(inputs["wk"], dtype=np.float32)
    wv = np.asarray(inputs["wv"], dtype=np.float32)
    bq = np.asarray(inputs["bq"], dtype=np.float32)
    bk = np.asarray(inputs["bk"], dtype=np.float32)
    bv = np.asarray(inputs["bv"], dtype=np.float32)

    if (mask != 1).any() or bq.any() or bk.any() or bv.any():
        return _numpy_reference(x_from, x_to, mask, wq, bq, wk, bk, wv, bv), None

    from concourse.bass_utils import run_bass_kernel_spmd

    nc = _get_nc()
    in_maps = _make_in_maps(x_from, x_to, wq, wk, wv)
    res = run_bass_kernel_spmd(nc, in_maps, list(range(NCORES)), **spmd_kwargs)
    return _assemble(res.results), res


def kernel(**inputs) -> np.ndarray:
    out, _ = _run(inputs)
    return out


def kernel_traced(**inputs):
    """Like kernel() but also returns the BassKernelResults (with
    exec_time_ns / profile when NTFF tracing is available)."""
    return _run(inputs, trace=True)



# revision 30
# speedup vs baseline: 1.1399x; 1.1399x over previous
"""Trainium2 Bass kernel for a dense multi-head attention layer.

Problem: B=4, S=2048, D=1024, H=16, DH=64 attention (QKV projections +
softmax(QK^T/sqrt(DH))V), fp32 reference, attention_mask all-ones, zero
biases.

Sharding (8 NeuronCores): core c handles batch b=c//2 and head-half
hh=c%2 (8 of 16 heads).  Per-core work is perfectly balanced with no
collectives: each core projects its 8 heads' Q/K/V over the full
sequence of its batch and runs attention for those heads.

Per-core device algorithm (all matmuls bf16 in / fp32 PSUM accumulate):
  - Q^T, K^T computed in [outcol, token] layout (lhsT = W, rhs = X^T).
  - V computed in [token, outcol] layout (lhsT = X^T tile, rhs = W),
    stored per (t_tile, head) with a constant ones column appended.
  - scores^T[t, f] per head via lhsT=K^T tile (contraction dh=64),
    heads sequential (measured: tile_position row-packing concurrency
    was a net loss on HW).
  - exp via ScalarE activation (scale=1/8 fused) straight out of PSUM,
    3 score tiles (both heads mixed, subtile-linear order) per
    activation to amortize the ~0.5us per-instruction ScalarE overhead
    -> bf16 expT in SBUF.  No max-subtraction (scores are O(1);
    softmax is shift-invariant).
  - PV: ctx^T[dh,f] = sum_t V[t,dh]*expT[t,f] with lhsT=[V|1] (M=65);
    row 64 accumulates the softmax denominator for free.
  - normalize: reciprocal of the denominator row, DRAM-bounce broadcast
    across 64 partitions, DVE multiply; DMA out as ctx^T [512, 2048].
Host reassembles: out[b, :, hh*512:(hh+1)*512] = core_out.T
"""

import numpy as np
import ml_dtypes

B, S, D = 4, 2048, 1024
H, DH = 16, 64
NCORES = 8
HL = 8            # local heads per core
OC = HL * DH      # 512 local output columns
P = 128
NDC = D // P      # 8 contraction chunks for projections
FB = 512          # f-block (query) width
NFB = S // FB     # 4
NTT = S // P      # 16 key tiles
SCALE = 1.0 / np.sqrt(DH)

_CACHE = {}


def _build_nc(repeat=None, variant=None):
    import contextlib
    import concourse.bass as bass
    import concourse.tile as tile
    from concourse import bacc, mybir
    from concourse.bass import ts, ds

    bf16 = mybir.dt.bfloat16
    f32 = mybir.dt.float32
    Exp = mybir.ActivationFunctionType.Exp

    nc = bacc.Bacc("TRN2", target_bir_lowering=False, debug=False)

    xfT_d = nc.dram_tensor("xfT", [D, S], bf16, kind="ExternalInput")
    xtT_d = nc.dram_tensor("xtT", [D, S], bf16, kind="ExternalInput")
    wq_d = nc.dram_tensor("wq", [D, OC], bf16, kind="ExternalInput")
    wk_d = nc.dram_tensor("wk", [D, OC], bf16, kind="ExternalInput")
    wv_d = nc.dram_tensor("wv", [D, OC], bf16, kind="ExternalInput")
    out_d = nc.dram_tensor("out", [S, OC], f32, kind="ExternalOutput")

    # groups of t_tiles per ScalarE activation, within each 8-tile half
    HGROUPS = [(0, 2), (2, 2), (4, 2), (6, 2)]

    with tile.TileContext(nc) as tc:
        with (
            tc.tile_pool(name="persist", bufs=1) as pp,
            tc.tile_pool(name="proj_in", bufs=1) as pin,
            tc.tile_pool(name="expt", bufs=1 if variant in ("pvonly", "pvnn") else 3) as ep,
            tc.tile_pool(name="small", bufs=2) as sp,
            tc.tile_pool(name="ps_sc2", bufs=2, space="PSUM") as ps_sc2,
            tc.tile_pool(name="ps_pv", bufs=2, space="PSUM") as ps_pv,
            tc.tile_pool(name="ps_b1", bufs=2, space="PSUM") as ps_b1,
        ):
            qT = pp.tile([P, 4, S], bf16, tag="qT")
            kT = pp.tile([P, 4, S], bf16, tag="kT")
            v = pp.tile([P, NTT, HL, DH + 1], bf16, tag="v")
            nc.vector.memset(v[:, :, :, DH], 1.0)
            ones64 = pp.tile([1, DH], f32, tag="ones64")
            nc.vector.memset(ones64[:], 1.0)
            eC = None
            if variant in ("pvonly", "pvnn"):
                eC = pp.tile([P, 16, FB], bf16, tag="eC")
                nc.vector.memset(eC[:], 0.001)

            xfT = pin.tile([P, NDC, S], bf16, tag="xfT")
            xtT = pin.tile([P, NDC, S], bf16, tag="xtT")
            wq = pin.tile([P, NDC, OC], bf16, tag="wq")
            wk = pin.tile([P, NDC, OC], bf16, tag="wk")
            wv = pin.tile([P, NDC, OC], bf16, tag="wv")
            def dma_w(sb_t, dr):
                nc.sync.dma_start(
                    out=sb_t[:],
                    in_=dr.ap().rearrange("(c p) n -> p c n", p=P),
                )

            def dma_x(sb_t, dr, tch):
                nc.sync.dma_start(
                    out=sb_t[:, :, ts(tch, FB)],
                    in_=dr.ap()[:, ts(tch, FB)].rearrange(
                        "(c p) n -> p c n", p=P),
                )

            # input DMAs chunked and ordered by first use so the prefix
            # projection chains start ~6us in instead of waiting ~31us for
            # the whole 11MB load.
            dma_w(wk, wk_d)
            dma_x(xtT, xtT_d, 0)
            dma_x(xtT, xtT_d, 1)
            dma_w(wq, wq_d)
            dma_x(xfT, xfT_d, 0)
            dma_w(wv, wv_d)
            dma_x(xtT, xtT_d, 2)
            dma_x(xtT, xtT_d, 3)
            dma_x(xfT, xfT_d, 1)
            dma_x(xfT, xfT_d, 2)
            dma_x(xfT, xfT_d, 3)

            def proj_chain(w_sb, x_sb, dst, ot, tch):
                psq = ps_b1.tile([P, FB], f32, tag="b1")
                for dc in range(NDC):
                    nc.tensor.matmul(
                        psq[:],
                        w_sb[:, dc, ts(ot, P)],
                        x_sb[:, dc, ts(tch, FB)],
                        start=(dc == 0),
                        stop=(dc == NDC - 1),
                    )
                nc.vector.tensor_copy(dst[:, ot, ts(tch, FB)], psq[:])

            def Kc(ot, tch):
                return lambda: proj_chain(wk, xtT, kT, ot, tch)

            def Qc(ot, tch):
                return lambda: proj_chain(wq, xfT, qT, ot, tch)

            def proj_v_one(tt):
                psv = ps_b1.tile([P, FB], f32, tag="b1")
                for dc in range(NDC):
                    nc.tensor.matmul(
                        psv[:],
                        xtT[:, dc, ts(tt, P)],
                        wv[:, dc, :],
                        start=(dc == 0),
                        stop=(dc == NDC - 1),
                    )
                nc.vector.tensor_copy(
                    v[:, tt, :, 0:DH],
                    psv[:].rearrange("p (h d) -> p h d", h=HL),
                )

            def Vc(tt):
                return lambda: proj_v_one(tt)

            def scores_half(j, fb, half, e):
                """scores^T + exp for t_tiles [8*half, 8*half+8), both heads
                of pair j, emitted strictly head-alternating [A0 B0 A1 B1
                ...] so consecutive MMs occupy different PE row groups
                (64x128 row tiling -> 2 concurrent MMs) and different PSUM
                banks; one ScalarE activation per 3-subtile slot."""
                if variant in ("pvonly", "pvnn"):
                    return
                order = []
                for q in range(4):
                    order += [(0, 2 * q), (0, 2 * q + 1),
                              (1, 2 * q), (1, 2 * q + 1)]
                bounds = [(2 * g, 2, ps_sc2) for g in range(8)]
                for gi, (start_s, glen, pool) in enumerate(bounds):
                    sc = pool.tile([P, glen, FB], f32, tag="sc")
                    for t in range(glen):
                        s = start_s + t
                        if variant == "quad":
                            hh_, i = order[s]
                        else:
                            hh_, i = s % 2, s // 2
                        tt = half * 8 + i
                        base = 0 if variant == "packoff" else hh_ * 64
                        if variant != "noscores":
                            nc.tensor.matmul(
                                sc[:, t, :],
                                kT[base:base + 64, j, ts(tt, P)],
                                qT[base:base + 64, j, ts(fb, FB)],
                                start=True, stop=True,
                                tile_position=(base, 0),
                            )
                    if variant == "half_act" and gi % 2:
                        continue
                    if variant in ("noact", "noscores"):
                        continue
                    nc.scalar.activation(
                        e[:, start_s:start_s + glen, :], sc[:, 0:glen, :],
                        Exp, scale=float(SCALE),
                    )

            def pv_full(cpsA, cpsB, j, e0, e1):
                """PV in ctx[f, dh] layout: lhsT = expT tile (128 cols ->
                FWL), rhs = [V|1] (streams 65 cols); psum col 64 of each
                f-tile block accumulates the softmax denominator, which
                lands per-PARTITION (f), so normalize is a native
                tensor_scalar.  The 4 f-tile chains share a PSUM bank, and
                start=True clears has_written BANK-wide, so each chain's 16
                MMs must run contiguously; A/B heads alternate banks."""
                if variant in ("nopv", "noact", "noscores"):
                    return
                if variant in ("pvonly", "pvnn"):
                    e0 = e1 = eC
                for ft in range(4):
                    for tt in range(NTT):
                        e = e0 if tt < 8 else e1
                        i = tt % 8
                        for hh_, cps in ((0, cpsA), (1, cpsB)):
                            nc.tensor.matmul(
                                cps[:, ft, :],
                                e[:, 2 * i + hh_, ts(ft, P)],
                                v[:, tt, 2 * j + hh_, :],
                                start=(tt == 0),
                                stop=(tt == NTT - 1),
                            )

            def norm_flip(cps, hl, fb):
                """Normalize ctx[f, dh] by the per-partition denominator
                (psum col 64 of each f-tile block) and DMA out in the
                natural [f, head*64+dh] layout."""
                if variant in ("nopv", "noact", "noscores", "pvnn"):
                    return
                recip = sp.tile([P, 4], f32, tag="recip")
                nc.vector.reciprocal(recip[:], cps[:, :, DH])
                outst = sp.tile([P, 4, DH], f32, tag="outst")
                for ft in range(4):
                    nc.vector.tensor_scalar_mul(
                        outst[:, ft, :], cps[:, ft, 0:DH],
                        recip[:, ft:ft + 1])
                nc.sync.dma_start(
                    out=out_d.ap()[ts(fb, FB), ds(hl * DH, DH)].rearrange(
                        "(ft p) d -> p ft d", p=P),
                    in_=outst[:],
                )

            rep_ctx = (
                tc.For_i(0, repeat, 1) if repeat else contextlib.nullcontext()
            )

            def finish_round(prev):
                """PV + normalize of the previous round -- emitted after
                the NEXT round's half-0 scores so ScalarE always has fresh
                score groups while TensorE runs the PV block (round-
                granularity software pipeline)."""
                e0, e1, j, fb = prev
                cpsA = ps_pv.tile([P, 4, DH + 1], f32, tag="pv")
                cpsB = ps_pv.tile([P, 4, DH + 1], f32, tag="pv")
                pv_full(cpsA, cpsB, j, e0, e1)
                norm_flip(cpsA, 2 * j, fb)
                norm_flip(cpsB, 2 * j + 1, fb)

            def attn_round(j, fb, prev, f0=(), f1=(), pre_scored=None):
                """One (pair, f-block) round, software-pipelined: emits
                half-0 scores, the PREVIOUS round's second-half PV+norm,
                half-0 PV, half-1 scores; returns the pending second half.
                f0 fillers run on TensorE while ScalarE drains the half-0
                exps; fend fillers at round end.  Fillers must not be placed
                between the cpsA alloc and the last norm (ps_b1 ring)."""
                if prev is not None and variant == "pvfirst":
                    finish_round(prev)
                if pre_scored is None:
                    e0 = ep.tile([P, 16, FB], bf16, tag="e")
                    scores_half(j, fb, 0, e0)
                else:
                    e0 = pre_scored
                if prev is not None and variant != "pvfirst":
                    finish_round(prev)
                for f in f0:
                    f()
                e1 = ep.tile([P, 16, FB], bf16, tag="e")
                scores_half(j, fb, 1, e1)
                for f in f1:
                    f()
                return (e0, e1, j, fb)

            # emission order = program order for Tile's dependency tracking,
            # and also the scheduling priority.  Get ScalarE started as early
            # as possible (scores of pair 0 need only qT/kT tile 0), then
            # spread the projection chains across the rounds so TensorE has
            # filler work exactly where it would otherwise stall on the
            # ScalarE exp drain (sc-pool buffers).
            with rep_ctx:
                # minimal prefix for scores(0,0,half0): kT tile0 t-chunks
                # 0-1 and qT tile0 chunk 0
                proj_chain(wk, xtT, kT, 0, 0)
                proj_chain(wk, xtT, kT, 0, 1)
                proj_chain(wq, xfT, qT, 0, 0)
                e00 = ep.tile([P, 16, FB], bf16, tag="e")
                scores_half(0, 0, 0, e00)
                proj_chain(wk, xtT, kT, 0, 2)
                proj_chain(wk, xtT, kT, 0, 3)
                # all fillers sit at the f1 slot (emitted AFTER the round's
                # scores) so they rank BELOW scores in TensorE priority and
                # only run when scores are blocked on the sc ring -- keeps
                # ScalarE fed.  Emission must still precede each chain's
                # consumer round (dependency correctness).
                f1s = {
                    (0, 0): [Qc(0, 1)] + [Vc(t) for t in range(16)],
                    (3, 3): [],
                }
                for j in range(4):
                    if j < 3:
                        f1s[(j, 1)] = [Kc(j + 1, 0), Qc(j, 2)]
                        f1s[(j, 2)] = [Kc(j + 1, 1), Qc(j, 3)]
                        f1s[(j, 3)] = [Qc(j + 1, 0), Kc(j + 1, 2),
                                       Kc(j + 1, 3), Qc(j + 1, 1)]
                        f1s.setdefault((j + 1, 0), [])
                    else:
                        f1s.setdefault((j, 1), [Qc(j, 2)])
                        f1s.setdefault((j, 2), [Qc(j, 3)])
                prev = None
                for j in range(4):
                    for fb in range(NFB):
                        prev = attn_round(
                            j, fb, prev, f1=f1s[(j, fb)],
                            pre_scored=e00 if (j, fb) == (0, 0) else None,
                        )
                finish_round(prev)
    nc.compile()
    return nc


def _get_nc():
    if "nc" not in _CACHE:
        _CACHE["nc"] = _build_nc()
    return _CACHE["nc"]


def _numpy_reference(x_from, x_to, attention_mask, wq, bq, wk, bk, wv, bv):
    """General fallback (used only if mask/biases are not the expected
    all-ones / zeros of this problem instance)."""
    b, fs, _ = x_from.shape
    ts_ = x_to.shape[1]
    q = (x_from @ wq + bq).reshape(b, fs, H, DH).transpose(0, 2, 1, 3)
    k = (x_to @ wk + bk).reshape(b, ts_, H, DH).transpose(0, 2, 1, 3)
    v = (x_to @ wv + bv).reshape(b, ts_, H, DH).transpose(0, 2, 1, 3)
    scores = np.einsum("bhfd,bhtd->bhft", q, k) * (1.0 / np.sqrt(DH))
    adder = (1.0 - attention_mask[:, None, :, :].astype(np.float32)) * -10000.0
    scores = scores + adder
    scores -= scores.max(axis=-1, keepdims=True)
    e = np.exp(scores)
    probs = e / e.sum(axis=-1, keepdims=True)
    ctx = np.einsum("bhft,bhtd->bhfd", probs, v)
    return ctx.transpose(0, 2, 1, 3).reshape(b, fs, H * DH).astype(np.float32)


def _make_in_maps(x_from, x_to, wq, wk, wv):
    bf = ml_dtypes.bfloat16
    xfT = [np.ascontiguousarray(x_from[b].T).astype(bf) for b in range(B)]
    xtT = [np.ascontiguousarray(x_to[b].T).astype(bf) for b in range(B)]
    wq_h = [np.ascontiguousarray(wq[:, hh * OC:(hh + 1) * OC]).astype(bf)
            for hh in range(2)]
    wk_h = [np.ascontiguousarray(wk[:, hh * OC:(hh + 1) * OC]).astype(bf)
            for hh in range(2)]
    wv_h = [np.ascontiguousarray(wv[:, hh * OC:(hh + 1) * OC]).astype(bf)
            for hh in range(2)]
    in_maps = []
    for c in range(NCORES):
        b, hh = c // 2, c % 2
        in_maps.append({
            "xfT": xfT[b], "xtT": xtT[b],
            "wq": wq_h[hh], "wk": wk_h[hh], "wv": wv_h[hh],
        })
    return in_maps


def _assemble(results):
    out = np.empty((B, S, H * DH), np.float32)
    for c in range(NCORES):
        b, hh = c // 2, c % 2
        out[b, :, hh * OC:(hh + 1) * OC] = results[c]["out"]
    return out


def _run(inputs, **spmd_kwargs):
    x_from = np.asarray(inputs["x_from"], dtype=np.float32)
    x_to = np.asarray(inputs["x_to"], dtype=np.float32)
    mask = np.asarray(inputs["attention_mask"])
    wq = np.asarray(inputs["wq"], dtype=np.float32)
    wk = np.asarray(inputs["wk"], dtype=np.float32)
    wv = np.asarray(inputs["wv"], dtype=np.float32)
    bq = np.asarray(inputs["bq"], dtype=np.float32)
    bk = np.asarray(inputs["bk"], dtype=np.float32)
    bv = np.asarray(inputs["bv"], dtype=np.float32)

    if (mask != 1).any() or bq.any() or bk.any() or bv.any():
        return _numpy_reference(x_from, x_to, mask, wq, bq, wk, bk, wv, bv), None

    from concourse.bass_utils import run_bass_kernel_spmd

    nc = _get_nc()
    in_maps = _make_in_maps(x_from, x_to, wq, wk, wv)
    res = run_bass_kernel_spmd(nc, in_maps, list(range(NCORES)), **spmd_kwargs)
    return _assemble(res.results), res


def kernel(**inputs) -> np.ndarray:
    out, _ = _run(inputs)
    return out


def kernel_traced(**inputs):
    """Like kernel() but also returns the BassKernelResults (with
    exec_time_ns / profile when NTFF tracing is available)."""
    return _run(inputs, trace=True)

